# revision 2
# baseline (speedup 1.0000x reference)
"""MoE routing kernel (nn_MoE_52037823758984) for 8x Trainium2 NeuronCores.

Computes out[i] = expert_{route[i]}(x[i]) where each expert is a Linear(10,10):
    y0 = x @ W1.T + b1 ; y1 = x @ W2.T + b2 ; out = where(route==0, y0, y1)

Sharding: data-parallel over the token dim. Each of the 8 cores processes
N/8 = 262144 tokens; the tiny 10x10 weights are baked into the program as
scalar immediates (the program is built at call time, when weights are known).

Baseline algorithm (token-major, f32-exact):
    out = y0 + r * (x @ Wd.T + bd)     with Wd = W2-W1, bd = b2-b1, r = route
        = (x*r) @ Wd.T + r*bd + x @ W1.T + b1     (by linearity)
  - x tile [128, R, 10]: partition = token block, free = (tokens, features)
  - xm = x * r (feature-broadcast of r via 10 tensor_tensor muls)
  - one accumulator `acc`; per output feature j:
      acc_j = xm_0*Wd[j,0] + b1[j]                  (tensor_scalar init)
      acc_j += xm_k*Wd[j,k]  k=1..9                 (scalar_tensor_tensor)
      acc_j += r*bd[j]                              (scalar_tensor_tensor)
      acc_j += x_k*W1[j,k]   k=0..9                 (scalar_tensor_tensor)
"""

import numpy as np

import concourse.bacc as bacc
import concourse.mybir as mybir
from concourse.tile import TileContext
from concourse.bass_utils import run_bass_kernel_spmd

F32 = mybir.dt.float32
I32 = mybir.dt.int32
ALU = mybir.AluOpType

N_CORES = 8
P = 128


def build_moe(tc_tokens, W1, b1, W2, b2, r_tile=256, reps=1):
    """Build + compile the per-core program for a shard of `tc_tokens` tokens."""
    D = 10
    Wd = (W2.astype(np.float64) - W1.astype(np.float64))
    bd = (b2.astype(np.float64) - b1.astype(np.float64))
    W1 = W1.astype(np.float64)
    b1 = b1.astype(np.float64)

    R = r_tile
    assert tc_tokens % (P * R) == 0
    nt = tc_tokens // (P * R)

    nc = bacc.Bacc("TRN2", target_bir_lowering=False, debug=False,
                   num_devices=N_CORES)
    x_ext = nc.dram_tensor("x", [tc_tokens, D], F32, kind="ExternalInput")
    r_ext = nc.dram_tensor("route", [tc_tokens], I32, kind="ExternalInput")
    o_ext = nc.dram_tensor("out", [tc_tokens, D], F32, kind="ExternalOutput")

    # partition p holds a contiguous run of R tokens
    xv = x_ext.rearrange("(n p r) d -> n p r d", p=P, r=R)
    rv = r_ext.rearrange("(n p r) -> n p r", p=P, r=R)
    ov = o_ext.rearrange("(n p r) d -> n p r d", p=P, r=R)

    with TileContext(nc) as tc:
        with tc.tile_pool(name="sbuf", bufs=2) as pool:
            for _ in range(reps):
                for i in range(nt):
                    xt = pool.tile([P, R, D], F32, tag="xt")
                    rt = pool.tile([P, R], I32, tag="rt")
                    nc.sync.dma_start(out=xt[:], in_=xv[i])
                    nc.sync.dma_start(out=rt[:], in_=rv[i])

                    rf = pool.tile([P, R], F32, tag="rf")
                    nc.vector.tensor_copy(out=rf[:], in_=rt[:])  # int->float

                    xm = pool.tile([P, R, D], F32, tag="xm")  # x * r
                    for k in range(D):
                        nc.vector.tensor_mul(out=xm[:, :, k], in0=xt[:, :, k],
                                             in1=rf[:])

                    acc = pool.tile([P, R, D], F32, tag="acc")
                    for j in range(D):
                        nc.vector.tensor_scalar(
                            out=acc[:, :, j], in0=xm[:, :, 0],
                            scalar1=float(Wd[j, 0]), scalar2=float(b1[j]),
                            op0=ALU.mult, op1=ALU.add)
                        for k in range(1, D):
                            nc.vector.scalar_tensor_tensor(
                                out=acc[:, :, j], in0=xm[:, :, k],
                                scalar=float(Wd[j, k]), in1=acc[:, :, j],
                                op0=ALU.mult, op1=ALU.add)
                        nc.vector.scalar_tensor_tensor(
                            out=acc[:, :, j], in0=rf[:],
                            scalar=float(bd[j]), in1=acc[:, :, j],
                            op0=ALU.mult, op1=ALU.add)
                        for k in range(D):
                            nc.vector.scalar_tensor_tensor(
                                out=acc[:, :, j], in0=xt[:, :, k],
                                scalar=float(W1[j, k]), in1=acc[:, :, j],
                                op0=ALU.mult, op1=ALU.add)
                    nc.sync.dma_start(out=ov[i], in_=acc[:])
    nc.compile()
    return nc


def run_sharded(nc, x, route, tc_tokens, out_name="out"):
    in_maps = []
    for c in range(N_CORES):
        sl = slice(c * tc_tokens, (c + 1) * tc_tokens)
        in_maps.append({"x": np.ascontiguousarray(x[sl]),
                        "route": np.ascontiguousarray(route[sl])})
    res = run_bass_kernel_spmd(nc, in_maps, core_ids=list(range(N_CORES)))
    return np.concatenate([res.results[c][out_name] for c in range(N_CORES)],
                          axis=0)


def kernel(x, W1, b1, W2, b2, route):
    x = np.asarray(x)
    route = np.asarray(route)
    tc_tokens = x.shape[0] // N_CORES
    nc = build_moe(tc_tokens, np.asarray(W1), np.asarray(b1),
                   np.asarray(W2), np.asarray(b2))
    return run_sharded(nc, x, route, tc_tokens)


# revision 14
# speedup vs baseline: 5.2502x; 5.2502x over previous
"""MoE routing kernel (nn_MoE_52037823758984) for 8x Trainium2 NeuronCores.

Computes out[i] = expert_{route[i]}(x[i]) where each expert is a Linear(10,10):
    y0 = x @ W1.T + b1 ; y1 = x @ W2.T + b2 ; out = where(route==0, y0, y1)

Sharding: data-parallel over the token dim. Each of the 8 cores processes
N/8 = 262144 tokens; the tiny 10x10 weights are baked into the program as
scalar immediates (the program is built at call time, when weights are known).

Baseline algorithm (token-major, f32-exact):
    out = y0 + r * (x @ Wd.T + bd)     with Wd = W2-W1, bd = b2-b1, r = route
        = (x*r) @ Wd.T + r*bd + x @ W1.T + b1     (by linearity)
  - x tile [128, R, 10]: partition = token block, free = (tokens, features)
  - xm = x * r (feature-broadcast of r via 10 tensor_tensor muls)
  - one accumulator `acc`; per output feature j:
      acc_j = xm_0*Wd[j,0] + b1[j]                  (tensor_scalar init)
      acc_j += xm_k*Wd[j,k]  k=1..9                 (scalar_tensor_tensor)
      acc_j += r*bd[j]                              (scalar_tensor_tensor)
      acc_j += x_k*W1[j,k]   k=0..9                 (scalar_tensor_tensor)
"""

import numpy as np

import concourse.bacc as bacc
import concourse.mybir as mybir
from concourse.tile import TileContext
from concourse.bass_utils import run_bass_kernel_spmd

F32 = mybir.dt.float32
I32 = mybir.dt.int32
ALU = mybir.AluOpType

N_CORES = 8
P = 128


def build_moe(tc_tokens, W1, b1, W2, b2, r_tile=256, reps=1):
    """Build + compile the per-core program for a shard of `tc_tokens` tokens."""
    D = 10
    Wd = (W2.astype(np.float64) - W1.astype(np.float64))
    bd = (b2.astype(np.float64) - b1.astype(np.float64))
    W1 = W1.astype(np.float64)
    b1 = b1.astype(np.float64)

    R = r_tile
    assert tc_tokens % (P * R) == 0
    nt = tc_tokens // (P * R)

    nc = bacc.Bacc("TRN2", target_bir_lowering=False, debug=False,
                   num_devices=N_CORES)
    x_ext = nc.dram_tensor("x", [tc_tokens, D], F32, kind="ExternalInput")
    r_ext = nc.dram_tensor("route", [tc_tokens], I32, kind="ExternalInput")
    o_ext = nc.dram_tensor("out", [tc_tokens, D], F32, kind="ExternalOutput")

    # partition p holds a contiguous run of R tokens
    xv = x_ext.rearrange("(n p r) d -> n p r d", p=P, r=R)
    rv = r_ext.rearrange("(n p r) -> n p r", p=P, r=R)
    ov = o_ext.rearrange("(n p r) d -> n p r d", p=P, r=R)

    with TileContext(nc) as tc:
        with tc.tile_pool(name="sbuf", bufs=2) as pool:
            for _ in range(reps):
                for i in range(nt):
                    xt = pool.tile([P, R, D], F32, tag="xt")
                    rt = pool.tile([P, R], I32, tag="rt")
                    nc.sync.dma_start(out=xt[:], in_=xv[i])
                    nc.sync.dma_start(out=rt[:], in_=rv[i])

                    rf = pool.tile([P, R], F32, tag="rf")
                    nc.vector.tensor_copy(out=rf[:], in_=rt[:])  # int->float

                    xm = pool.tile([P, R, D], F32, tag="xm")  # x * r
                    for k in range(D):
                        nc.vector.tensor_mul(out=xm[:, :, k], in0=xt[:, :, k],
                                             in1=rf[:])

                    acc = pool.tile([P, R, D], F32, tag="acc")
                    for j in range(D):
                        nc.vector.tensor_scalar(
                            out=acc[:, :, j], in0=xm[:, :, 0],
                            scalar1=float(Wd[j, 0]), scalar2=float(b1[j]),
                            op0=ALU.mult, op1=ALU.add)
                        for k in range(1, D):
                            nc.vector.scalar_tensor_tensor(
                                out=acc[:, :, j], in0=xm[:, :, k],
                                scalar=float(Wd[j, k]), in1=acc[:, :, j],
                                op0=ALU.mult, op1=ALU.add)
                        nc.vector.scalar_tensor_tensor(
                            out=acc[:, :, j], in0=rf[:],
                            scalar=float(bd[j]), in1=acc[:, :, j],
                            op0=ALU.mult, op1=ALU.add)
                        for k in range(D):
                            nc.vector.scalar_tensor_tensor(
                                out=acc[:, :, j], in0=xt[:, :, k],
                                scalar=float(W1[j, k]), in1=acc[:, :, j],
                                op0=ALU.mult, op1=ALU.add)
                    nc.sync.dma_start(out=ov[i], in_=acc[:])
    nc.compile()
    return nc


def build_moe_planar(tc_tokens, W1, b1, W2, b2, r_tile=256, reps=1):
    """Planar variant: all DVE ops on contiguous [128, R] slices; weights as
    [128,1] SBUF scalars (replicated via a small extra input) instead of
    per-instruction immediates.

    wt layout (cols): 0-99 Wd[j,k] at j*10+k; 100-199 W1[j,k]; 200-209 bd;
    210-219 b1.
    """
    D = 10
    R = r_tile
    assert tc_tokens % (P * R) == 0
    nt = tc_tokens // (P * R)

    nc = bacc.Bacc("TRN2", target_bir_lowering=False, debug=False,
                   num_devices=N_CORES)
    x_ext = nc.dram_tensor("x", [tc_tokens, D], F32, kind="ExternalInput")
    r_ext = nc.dram_tensor("route", [tc_tokens], I32, kind="ExternalInput")
    w_ext = nc.dram_tensor("wt", [P, 220], F32, kind="ExternalInput")
    o_ext = nc.dram_tensor("out", [tc_tokens, D], F32, kind="ExternalOutput")

    xv = x_ext.rearrange("(n p r) d -> n p r d", p=P, r=R)
    rv = r_ext.rearrange("(n p r) -> n p r", p=P, r=R)
    ov = o_ext.rearrange("(n p r) d -> n p r d", p=P, r=R)

    with TileContext(nc) as tc:
        with tc.tile_pool(name="const", bufs=1) as cpool, \
             tc.tile_pool(name="sbuf", bufs=2) as pool:
            wt = cpool.tile([P, 220], F32)
            nc.sync.dma_start(out=wt[:], in_=w_ext[:])

            def wd(j, k):
                return wt[:, j * 10 + k:j * 10 + k + 1]

            def w1(j, k):
                return wt[:, 100 + j * 10 + k:100 + j * 10 + k + 1]

            def bd(j):
                return wt[:, 200 + j:200 + j + 1]

            def b1(j):
                return wt[:, 210 + j:210 + j + 1]

            for _ in range(reps):
                for i in range(nt):
                    xt = pool.tile([P, R, D], F32, tag="xt")
                    rt = pool.tile([P, R], I32, tag="rt")
                    nc.sync.dma_start(out=xt[:], in_=xv[i])
                    nc.sync.dma_start(out=rt[:], in_=rv[i])

                    rf = pool.tile([P, R], F32, tag="rf")
                    nc.vector.tensor_copy(out=rf[:], in_=rt[:])

                    xp = pool.tile([P, D, R], F32, tag="xp")  # planar x
                    for k in range(D):
                        nc.vector.tensor_copy(out=xp[:, k, :], in_=xt[:, :, k])

                    accp = pool.tile([P, D, R], F32, tag="accp")
                    for j in range(D):
                        aj = accp[:, j, :]
                        nc.vector.tensor_scalar(
                            out=aj, in0=xp[:, 0, :], scalar1=wd(j, 0),
                            scalar2=bd(j), op0=ALU.mult, op1=ALU.add)
                        for k in range(1, D):
                            nc.vector.scalar_tensor_tensor(
                                out=aj, in0=xp[:, k, :], scalar=wd(j, k),
                                in1=aj, op0=ALU.mult, op1=ALU.add)
                        # mask the delta expert, then add expert-1 terms
                        nc.vector.tensor_mul(out=aj, in0=aj, in1=rf[:])
                        for k in range(D):
                            nc.vector.scalar_tensor_tensor(
                                out=aj, in0=xp[:, k, :], scalar=w1(j, k),
                                in1=aj, op0=ALU.mult, op1=ALU.add)
                        nc.vector.tensor_scalar_add(out=aj, in0=aj,
                                                    scalar1=b1(j))
                    # un-planarize and store
                    acc = pool.tile([P, R, D], F32, tag="acc")
                    for d in range(D):
                        nc.vector.tensor_copy(out=acc[:, :, d], in_=accp[:, d, :])
                    nc.sync.dma_start(out=ov[i], in_=acc[:])
    nc.compile()
    return nc


def build_moe_v3(tc_tokens, W1, b1, W2, b2, r_tile=256, reps=1, gp_tiles=2):
    """v3: engine-split variant.

    - chain (the 210 multiply-accumulate ops/tile) runs on DVE for most tiles
      and on GPSIMD for `gp_tiles` of every 8, so the two engines work in
      parallel;
    - glue ops move to the Scalar engine (ACT): feature-planarize copies and
      the un-planarize which is fused with the per-feature bias add
      (ACTIVATE Copy with per-partition bias AP).
    """
    D = 10
    R = r_tile
    assert tc_tokens % (P * R) == 0
    nt = tc_tokens // (P * R)
    AF = mybir.ActivationFunctionType

    nc = bacc.Bacc("TRN2", target_bir_lowering=False, debug=False,
                   num_devices=N_CORES)
    x_ext = nc.dram_tensor("x", [tc_tokens, D], F32, kind="ExternalInput")
    r_ext = nc.dram_tensor("route", [tc_tokens], I32, kind="ExternalInput")
    w_ext = nc.dram_tensor("wt", [P, 220], F32, kind="ExternalInput")
    o_ext = nc.dram_tensor("out", [tc_tokens, D], F32, kind="ExternalOutput")

    xv = x_ext.rearrange("(n p r) d -> n p r d", p=P, r=R)
    rv = r_ext.rearrange("(n p r) -> n p r", p=P, r=R)
    ov = o_ext.rearrange("(n p r) d -> n p r d", p=P, r=R)

    # spread the gpsimd-chain tiles evenly through the loop
    gp_set = set()
    if gp_tiles > 0:
        stride = max(1, nt // gp_tiles)
        gp_set = {i for i in range(nt) if i % stride == stride - 1}
        while len(gp_set) > gp_tiles:
            gp_set.pop()

    with TileContext(nc) as tc:
        n_bufs = 3 if R <= 256 else 2
        with tc.tile_pool(name="const", bufs=1) as cpool, \
             tc.tile_pool(name="sbuf", bufs=n_bufs) as pool:
            wt = cpool.tile([P, 220], F32)
            nc.sync.dma_start(out=wt[:], in_=w_ext[:])

            def ap_wd(j, k):
                return wt[:, j * 10 + k:j * 10 + k + 1]

            def ap_w1(j, k):
                return wt[:, 100 + j * 10 + k:100 + j * 10 + k + 1]

            def ap_bd(j):
                return wt[:, 200 + j:200 + j + 1]

            def ap_b1(j):
                return wt[:, 210 + j:210 + j + 1]

            for _ in range(reps):
                for i in range(nt):
                    eng = nc.gpsimd if i in gp_set else nc.vector
                    xt = pool.tile([P, R, D], F32, tag="xt")
                    rt = pool.tile([P, R], I32, tag="rt")
                    nc.sync.dma_start(out=xt[:], in_=xv[i])
                    nc.sync.dma_start(out=rt[:], in_=rv[i])

                    rf = pool.tile([P, R], F32, tag="rf")
                    eng.tensor_copy(out=rf[:], in_=rt[:])

                    xp = pool.tile([P, D, R], F32, tag="xp")
                    for k in range(D):
                        nc.scalar.copy(out=xp[:, k, :], in_=xt[:, :, k])

                    is_gp = i in gp_set
                    Wdv = W2.astype(np.float64) - W1.astype(np.float64)
                    bdv = b2.astype(np.float64) - b1.astype(np.float64)

                    def s_wd(j, k):
                        return float(Wdv[j, k]) if is_gp else ap_wd(j, k)

                    def s_w1(j, k):
                        return float(W1[j, k]) if is_gp else ap_w1(j, k)

                    def s_bd(j):
                        return float(bdv[j]) if is_gp else ap_bd(j)

                    accp = pool.tile([P, D, R], F32, tag="accp")
                    if is_gp:
                        tmp = pool.tile([P, R], F32, tag="gptmp")
                    for j in range(D):
                        aj = accp[:, j, :]
                        if is_gp:
                            # Pool engine has no fused scalar_tensor_tensor;
                            # use mul + add pairs with float immediates.
                            eng.tensor_scalar_mul(out=aj, in0=xp[:, 0, :],
                                                  scalar1=s_wd(j, 0))
                            eng.tensor_scalar_add(out=aj, in0=aj,
                                                  scalar1=s_bd(j))
                            for k in range(1, D):
                                eng.tensor_scalar_mul(out=tmp[:], in0=xp[:, k, :],
                                                      scalar1=s_wd(j, k))
                                eng.tensor_add(out=aj, in0=aj, in1=tmp[:])
                            eng.tensor_mul(out=aj, in0=aj, in1=rf[:])
                            for k in range(D):
                                eng.tensor_scalar_mul(out=tmp[:], in0=xp[:, k, :],
                                                      scalar1=s_w1(j, k))
                                eng.tensor_add(out=aj, in0=aj, in1=tmp[:])
                        else:
                            eng.tensor_scalar(
                                out=aj, in0=xp[:, 0, :], scalar1=s_wd(j, 0),
                                scalar2=s_bd(j), op0=ALU.mult, op1=ALU.add)
                            for k in range(1, D):
                                eng.scalar_tensor_tensor(
                                    out=aj, in0=xp[:, k, :], scalar=s_wd(j, k),
                                    in1=aj, op0=ALU.mult, op1=ALU.add)
                            eng.tensor_mul(out=aj, in0=aj, in1=rf[:])
                            for k in range(D):
                                eng.scalar_tensor_tensor(
                                    out=aj, in0=xp[:, k, :], scalar=s_w1(j, k),
                                    in1=aj, op0=ALU.mult, op1=ALU.add)
                    # un-planarize fused with bias add on ACT
                    acc = pool.tile([P, R, D], F32, tag="acc")
                    for j in range(D):
                        nc.scalar.activation(out=acc[:, :, j], in_=accp[:, j, :],
                                             func=AF.Identity, bias=ap_b1(j),
                                             scale=1.0)
                    nc.sync.dma_start(out=ov[i], in_=acc[:])
    nc.compile()
    return nc


def make_wt(W1, b1, W2, b2):
    Wd = (W2 - W1)
    bdv = (b2 - b1)
    cols = np.concatenate([Wd.reshape(-1), W1.reshape(-1), bdv, b1]).astype(np.float32)
    return np.tile(cols[None, :], (P, 1))


def run_sharded(nc, x, route, tc_tokens, wt=None, out_name="out"):
    in_maps = []
    for c in range(N_CORES):
        sl = slice(c * tc_tokens, (c + 1) * tc_tokens)
        m = {"x": np.ascontiguousarray(x[sl]),
             "route": np.ascontiguousarray(route[sl])}
        if wt is not None:
            m["wt"] = wt
        in_maps.append(m)
    res = run_bass_kernel_spmd(nc, in_maps, core_ids=list(range(N_CORES)))
    return np.concatenate([res.results[c][out_name] for c in range(N_CORES)],
                          axis=0)


def kernel(x, W1, b1, W2, b2, route):
    x = np.asarray(x)
    route = np.asarray(route)
    W1, b1 = np.asarray(W1), np.asarray(b1)
    W2, b2 = np.asarray(W2), np.asarray(b2)
    tc_tokens = x.shape[0] // N_CORES
    # v3 with gp_tiles=0 == DVE chain + Scalar-engine glue; fastest measured
    nc = build_moe_v3(tc_tokens, W1, b1, W2, b2, gp_tiles=0)
    return run_sharded(nc, x, route, tc_tokens, wt=make_wt(W1, b1, W2, b2))


# revision 17
# speedup vs baseline: 7.1250x; 1.3571x over previous
"""MoE routing kernel (nn_MoE_52037823758984) for 8x Trainium2 NeuronCores.

Computes out[i] = expert_{route[i]}(x[i]) where each expert is a Linear(10,10):
    y0 = x @ W1.T + b1 ; y1 = x @ W2.T + b2 ; out = where(route==0, y0, y1)

Sharding: data-parallel over the token dim. Each of the 8 cores processes
N/8 = 262144 tokens; the tiny 10x10 weights are baked into the program as
scalar immediates (the program is built at call time, when weights are known).

Baseline algorithm (token-major, f32-exact):
    out = y0 + r * (x @ Wd.T + bd)     with Wd = W2-W1, bd = b2-b1, r = route
        = (x*r) @ Wd.T + r*bd + x @ W1.T + b1     (by linearity)
  - x tile [128, R, 10]: partition = token block, free = (tokens, features)
  - xm = x * r (feature-broadcast of r via 10 tensor_tensor muls)
  - one accumulator `acc`; per output feature j:
      acc_j = xm_0*Wd[j,0] + b1[j]                  (tensor_scalar init)
      acc_j += xm_k*Wd[j,k]  k=1..9                 (scalar_tensor_tensor)
      acc_j += r*bd[j]                              (scalar_tensor_tensor)
      acc_j += x_k*W1[j,k]   k=0..9                 (scalar_tensor_tensor)
"""

import numpy as np

import concourse.bacc as bacc
import concourse.mybir as mybir
from concourse.tile import TileContext
from concourse.bass_utils import run_bass_kernel_spmd

F32 = mybir.dt.float32
I32 = mybir.dt.int32
ALU = mybir.AluOpType

N_CORES = 8
P = 128


def build_moe(tc_tokens, W1, b1, W2, b2, r_tile=256, reps=1):
    """Build + compile the per-core program for a shard of `tc_tokens` tokens."""
    D = 10
    Wd = (W2.astype(np.float64) - W1.astype(np.float64))
    bd = (b2.astype(np.float64) - b1.astype(np.float64))
    W1 = W1.astype(np.float64)
    b1 = b1.astype(np.float64)

    R = r_tile
    assert tc_tokens % (P * R) == 0
    nt = tc_tokens // (P * R)

    nc = bacc.Bacc("TRN2", target_bir_lowering=False, debug=False,
                   num_devices=N_CORES)
    x_ext = nc.dram_tensor("x", [tc_tokens, D], F32, kind="ExternalInput")
    r_ext = nc.dram_tensor("route", [tc_tokens], I32, kind="ExternalInput")
    o_ext = nc.dram_tensor("out", [tc_tokens, D], F32, kind="ExternalOutput")

    # partition p holds a contiguous run of R tokens
    xv = x_ext.rearrange("(n p r) d -> n p r d", p=P, r=R)
    rv = r_ext.rearrange("(n p r) -> n p r", p=P, r=R)
    ov = o_ext.rearrange("(n p r) d -> n p r d", p=P, r=R)

    with TileContext(nc) as tc:
        with tc.tile_pool(name="sbuf", bufs=2) as pool:
            for _ in range(reps):
                for i in range(nt):
                    xt = pool.tile([P, R, D], F32, tag="xt")
                    rt = pool.tile([P, R], I32, tag="rt")
                    nc.sync.dma_start(out=xt[:], in_=xv[i])
                    nc.sync.dma_start(out=rt[:], in_=rv[i])

                    rf = pool.tile([P, R], F32, tag="rf")
                    nc.vector.tensor_copy(out=rf[:], in_=rt[:])  # int->float

                    xm = pool.tile([P, R, D], F32, tag="xm")  # x * r
                    for k in range(D):
                        nc.vector.tensor_mul(out=xm[:, :, k], in0=xt[:, :, k],
                                             in1=rf[:])

                    acc = pool.tile([P, R, D], F32, tag="acc")
                    for j in range(D):
                        nc.vector.tensor_scalar(
                            out=acc[:, :, j], in0=xm[:, :, 0],
                            scalar1=float(Wd[j, 0]), scalar2=float(b1[j]),
                            op0=ALU.mult, op1=ALU.add)
                        for k in range(1, D):
                            nc.vector.scalar_tensor_tensor(
                                out=acc[:, :, j], in0=xm[:, :, k],
                                scalar=float(Wd[j, k]), in1=acc[:, :, j],
                                op0=ALU.mult, op1=ALU.add)
                        nc.vector.scalar_tensor_tensor(
                            out=acc[:, :, j], in0=rf[:],
                            scalar=float(bd[j]), in1=acc[:, :, j],
                            op0=ALU.mult, op1=ALU.add)
                        for k in range(D):
                            nc.vector.scalar_tensor_tensor(
                                out=acc[:, :, j], in0=xt[:, :, k],
                                scalar=float(W1[j, k]), in1=acc[:, :, j],
                                op0=ALU.mult, op1=ALU.add)
                    nc.sync.dma_start(out=ov[i], in_=acc[:])
    nc.compile()
    return nc


def build_moe_planar(tc_tokens, W1, b1, W2, b2, r_tile=256, reps=1):
    """Planar variant: all DVE ops on contiguous [128, R] slices; weights as
    [128,1] SBUF scalars (replicated via a small extra input) instead of
    per-instruction immediates.

    wt layout (cols): 0-99 Wd[j,k] at j*10+k; 100-199 W1[j,k]; 200-209 bd;
    210-219 b1.
    """
    D = 10
    R = r_tile
    assert tc_tokens % (P * R) == 0
    nt = tc_tokens // (P * R)

    nc = bacc.Bacc("TRN2", target_bir_lowering=False, debug=False,
                   num_devices=N_CORES)
    x_ext = nc.dram_tensor("x", [tc_tokens, D], F32, kind="ExternalInput")
    r_ext = nc.dram_tensor("route", [tc_tokens], I32, kind="ExternalInput")
    w_ext = nc.dram_tensor("wt", [P, 220], F32, kind="ExternalInput")
    o_ext = nc.dram_tensor("out", [tc_tokens, D], F32, kind="ExternalOutput")

    xv = x_ext.rearrange("(n p r) d -> n p r d", p=P, r=R)
    rv = r_ext.rearrange("(n p r) -> n p r", p=P, r=R)
    ov = o_ext.rearrange("(n p r) d -> n p r d", p=P, r=R)

    with TileContext(nc) as tc:
        with tc.tile_pool(name="const", bufs=1) as cpool, \
             tc.tile_pool(name="sbuf", bufs=2) as pool:
            wt = cpool.tile([P, 220], F32)
            nc.sync.dma_start(out=wt[:], in_=w_ext[:])

            def wd(j, k):
                return wt[:, j * 10 + k:j * 10 + k + 1]

            def w1(j, k):
                return wt[:, 100 + j * 10 + k:100 + j * 10 + k + 1]

            def bd(j):
                return wt[:, 200 + j:200 + j + 1]

            def b1(j):
                return wt[:, 210 + j:210 + j + 1]

            for _ in range(reps):
                for i in range(nt):
                    xt = pool.tile([P, R, D], F32, tag="xt")
                    rt = pool.tile([P, R], I32, tag="rt")
                    nc.sync.dma_start(out=xt[:], in_=xv[i])
                    nc.sync.dma_start(out=rt[:], in_=rv[i])

                    rf = pool.tile([P, R], F32, tag="rf")
                    nc.vector.tensor_copy(out=rf[:], in_=rt[:])

                    xp = pool.tile([P, D, R], F32, tag="xp")  # planar x
                    for k in range(D):
                        nc.vector.tensor_copy(out=xp[:, k, :], in_=xt[:, :, k])

                    accp = pool.tile([P, D, R], F32, tag="accp")
                    for j in range(D):
                        aj = accp[:, j, :]
                        nc.vector.tensor_scalar(
                            out=aj, in0=xp[:, 0, :], scalar1=wd(j, 0),
                            scalar2=bd(j), op0=ALU.mult, op1=ALU.add)
                        for k in range(1, D):
                            nc.vector.scalar_tensor_tensor(
                                out=aj, in0=xp[:, k, :], scalar=wd(j, k),
                                in1=aj, op0=ALU.mult, op1=ALU.add)
                        # mask the delta expert, then add expert-1 terms
                        nc.vector.tensor_mul(out=aj, in0=aj, in1=rf[:])
                        for k in range(D):
                            nc.vector.scalar_tensor_tensor(
                                out=aj, in0=xp[:, k, :], scalar=w1(j, k),
                                in1=aj, op0=ALU.mult, op1=ALU.add)
                        nc.vector.tensor_scalar_add(out=aj, in0=aj,
                                                    scalar1=b1(j))
                    # un-planarize and store
                    acc = pool.tile([P, R, D], F32, tag="acc")
                    for d in range(D):
                        nc.vector.tensor_copy(out=acc[:, :, d], in_=accp[:, d, :])
                    nc.sync.dma_start(out=ov[i], in_=acc[:])
    nc.compile()
    return nc


def build_moe_v3(tc_tokens, W1, b1, W2, b2, r_tile=256, reps=1, gp_tiles=2,
                 layout="new"):
    """v3: engine-split variant.

    - chain (the 210 multiply-accumulate ops/tile) runs on DVE for most tiles
      and on GPSIMD for `gp_tiles` of every 8, so the two engines work in
      parallel;
    - glue ops move to the Scalar engine (ACT): feature-planarize copies and
      the un-planarize which is fused with the per-feature bias add
      (ACTIVATE Copy with per-partition bias AP).
    """
    D = 10
    R = r_tile
    assert tc_tokens % (P * R) == 0
    nt = tc_tokens // (P * R)
    AF = mybir.ActivationFunctionType

    nc = bacc.Bacc("TRN2", target_bir_lowering=False, debug=False,
                   num_devices=N_CORES)
    x_ext = nc.dram_tensor("x", [tc_tokens, D], F32, kind="ExternalInput")
    r_ext = nc.dram_tensor("route", [tc_tokens], I32, kind="ExternalInput")
    w_ext = nc.dram_tensor("wt", [P, 220], F32, kind="ExternalInput")
    o_ext = nc.dram_tensor("out", [tc_tokens, D], F32, kind="ExternalOutput")

    xv = x_ext.rearrange("(n p r) d -> n p r d", p=P, r=R)
    rv = r_ext.rearrange("(n p r) -> n p r", p=P, r=R)
    ov = o_ext.rearrange("(n p r) d -> n p r d", p=P, r=R)

    # spread the gpsimd-chain tiles evenly through the loop
    gp_set = set()
    if gp_tiles > 0:
        stride = max(1, nt // gp_tiles)
        gp_set = {i for i in range(nt) if i % stride == stride - 1}
        while len(gp_set) > gp_tiles:
            gp_set.pop()

    with TileContext(nc) as tc:
        n_bufs = (4 if R <= 256 else 3) if layout == 'new' else 3
        with tc.tile_pool(name="const", bufs=1) as cpool, \
             tc.tile_pool(name="sbuf", bufs=n_bufs) as pool:
            wt = cpool.tile([P, 220], F32)
            nc.sync.dma_start(out=wt[:], in_=w_ext[:])

            def ap_wd(j, k):
                return wt[:, j * 10 + k:j * 10 + k + 1]

            def ap_w1(j, k):
                return wt[:, 100 + j * 10 + k:100 + j * 10 + k + 1]

            def ap_bd(j):
                return wt[:, 200 + j:200 + j + 1]

            def ap_b1(j):
                return wt[:, 210 + j:210 + j + 1]

            for _ in range(reps):
                for i in range(nt):
                    eng = nc.gpsimd if i in gp_set else nc.vector
                    xt = pool.tile([P, R, D], F32, tag="xt")
                    rt = pool.tile([P, R], I32,
                                   tag="rtf" if layout == "new" else "rt")
                    nc.sync.dma_start(out=xt[:], in_=xv[i])
                    nc.sync.dma_start(out=rt[:], in_=rv[i])

                    rf = pool.tile([P, R], F32,
                                   tag="rtf" if layout == "new" else "rf")
                    eng.tensor_copy(out=rf[:], in_=rt[:])

                    xp = pool.tile([P, D, R], F32, tag="xp")
                    for k in range(D):
                        nc.scalar.copy(out=xp[:, k, :], in_=xt[:, :, k])

                    is_gp = i in gp_set
                    Wdv = W2.astype(np.float64) - W1.astype(np.float64)
                    bdv = b2.astype(np.float64) - b1.astype(np.float64)

                    def s_wd(j, k):
                        return float(Wdv[j, k]) if is_gp else ap_wd(j, k)

                    def s_w1(j, k):
                        return float(W1[j, k]) if is_gp else ap_w1(j, k)

                    def s_bd(j):
                        return float(bdv[j]) if is_gp else ap_bd(j)

                    accp = pool.tile([P, D, R], F32, tag="accp")
                    if is_gp:
                        tmp = pool.tile([P, R], F32, tag="gptmp")
                    for j in range(D):
                        aj = accp[:, j, :]
                        if is_gp:
                            # Pool engine has no fused scalar_tensor_tensor;
                            # use mul + add pairs with float immediates.
                            eng.tensor_scalar_mul(out=aj, in0=xp[:, 0, :],
                                                  scalar1=s_wd(j, 0))
                            eng.tensor_scalar_add(out=aj, in0=aj,
                                                  scalar1=s_bd(j))
                            for k in range(1, D):
                                eng.tensor_scalar_mul(out=tmp[:], in0=xp[:, k, :],
                                                      scalar1=s_wd(j, k))
                                eng.tensor_add(out=aj, in0=aj, in1=tmp[:])
                            eng.tensor_mul(out=aj, in0=aj, in1=rf[:])
                            for k in range(D):
                                eng.tensor_scalar_mul(out=tmp[:], in0=xp[:, k, :],
                                                      scalar1=s_w1(j, k))
                                eng.tensor_add(out=aj, in0=aj, in1=tmp[:])
                        else:
                            eng.tensor_scalar(
                                out=aj, in0=xp[:, 0, :], scalar1=s_wd(j, 0),
                                scalar2=s_bd(j), op0=ALU.mult, op1=ALU.add)
                            for k in range(1, D):
                                eng.scalar_tensor_tensor(
                                    out=aj, in0=xp[:, k, :], scalar=s_wd(j, k),
                                    in1=aj, op0=ALU.mult, op1=ALU.add)
                            eng.tensor_mul(out=aj, in0=aj, in1=rf[:])
                            for k in range(D):
                                eng.scalar_tensor_tensor(
                                    out=aj, in0=xp[:, k, :], scalar=s_w1(j, k),
                                    in1=aj, op0=ALU.mult, op1=ALU.add)
                    # un-planarize fused with bias add on ACT; reuse the
                    # xt ring (xt is dead once planarized)
                    acc = pool.tile([P, R, D], F32,
                                    tag="xt" if layout == "new" else "acc")
                    for j in range(D):
                        nc.scalar.activation(out=acc[:, :, j], in_=accp[:, j, :],
                                             func=AF.Identity, bias=ap_b1(j),
                                             scale=1.0)
                    nc.sync.dma_start(out=ov[i], in_=acc[:])
    nc.compile()
    return nc


def make_wt(W1, b1, W2, b2):
    Wd = (W2 - W1)
    bdv = (b2 - b1)
    cols = np.concatenate([Wd.reshape(-1), W1.reshape(-1), bdv, b1]).astype(np.float32)
    return np.tile(cols[None, :], (P, 1))


def run_sharded(nc, x, route, tc_tokens, wt=None, out_name="out"):
    in_maps = []
    for c in range(N_CORES):
        sl = slice(c * tc_tokens, (c + 1) * tc_tokens)
        m = {"x": np.ascontiguousarray(x[sl]),
             "route": np.ascontiguousarray(route[sl])}
        if wt is not None:
            m["wt"] = wt
        in_maps.append(m)
    res = run_bass_kernel_spmd(nc, in_maps, core_ids=list(range(N_CORES)))
    return np.concatenate([res.results[c][out_name] for c in range(N_CORES)],
                          axis=0)


def kernel(x, W1, b1, W2, b2, route):
    x = np.asarray(x)
    route = np.asarray(route)
    W1, b1 = np.asarray(W1), np.asarray(b1)
    W2, b2 = np.asarray(W2), np.asarray(b2)
    tc_tokens = x.shape[0] // N_CORES
    # v3 with gp_tiles=0 == DVE chain + Scalar-engine glue; fastest measured
    # config in same-process A/B: R=512 tiles, output staging reusing the xt
    # ring, bufs=3
    nc = build_moe_v3(tc_tokens, W1, b1, W2, b2, r_tile=512, gp_tiles=0)
    return run_sharded(nc, x, route, tc_tokens, wt=make_wt(W1, b1, W2, b2))
